# revision 1
# baseline (speedup 1.0000x reference)
"""Trainium2 Bass kernel for nn_Attention_3126736192307.

Causal multi-head attention with RoPE: B=2, S=2048, H=2048, 16 heads x 128.

Sharding (tensor parallel over heads, 8 cores, 2 heads each):
  - Wq/Wk/Wv column-split (per-head), Wo row-split; each core computes a
    partial [B*S, H] output; the host sums the 8 partials (row-parallel
    unshard) - no on-device collectives needed.

Per-core dataflow (all matmuls transpose-free by construction):
  - Host pre-transposes: X.T [H, T], WqT/WkT [H, 256] (head-dim permuted so
    RoPE's rotate_half becomes an intra-quadrant stream_shuffle), WvT [H, 256],
    WoT [256, H], cos/sin [128, T] feature-major (sin sign-folded).
  - Phase 1: q,k feature-major [128, T] per head + RoPE (DVE); v token-major.
  - Phase 2 per (b, h, i-chunk): scores.T [j,i] = k.T (lhsT) @ q.T; exp on
    ScalarE (no max subtraction - scores are ~N(0,1) after the 1/sqrt(hd)
    scale); causal block skipping + 0/1 mask multiply on diagonal-crossing
    tiles; column sums via ones-matmul on TensorE; AV accumulation in PSUM;
    normalization folded into the PSUM->SBUF eviction.
  - Phase 3: out.T (lhsT) @ WoT -> partial [T, H], streamed to DRAM.

Matmuls run in bf16 (1 PE cycle/row; fp32 is 4x, and fp32r's fused
weight-load encoding can't carry the 2 semaphore waits Tile emits).
"""

import os
import sys

for _p in ("/opt/trn_rl_repo", "/root/.axon_site/_ro/trn_rl_repo"):
    if os.path.isdir(_p) and _p not in sys.path:
        sys.path.append(_p)

from contextlib import ExitStack

import ml_dtypes
import numpy as np

import concourse.bass as bass
import concourse.bacc as bacc
import concourse.tile as tile
from concourse import mybir
from concourse.bass_utils import run_bass_kernel_spmd

B, S, H, NH = 2, 2048, 2048, 16
HD = 128
NCORES = 8
HPC = NH // NCORES            # heads per core = 2
M = HPC * HD                  # 256 output channels per core
SCALE = HD ** -0.5
P = 128                       # partitions
NKT = H // P                  # 16 contraction tiles for projections

F32 = mybir.dt.float32

# head-dim permutation: interleave halves at 16 granularity so the RoPE
# partner (d <-> d+64) sits 16 partitions away inside one 32-part quadrant
PERM = np.concatenate([np.arange(16 * m, 16 * m + 16) + (64 if odd else 0)
                       for m in range(4) for odd in (0, 1)])
SWAP_MASK = [i ^ 16 for i in range(32)]


BF16 = ml_dtypes.bfloat16


def build_masks(tchunk):
    """0/1 keep-masks for the R diagonal-crossing j-tiles of each i-chunk."""
    r = tchunk // P
    m = np.zeros((r, P, tchunk), np.float32)
    il = np.arange(tchunk)
    for ri in range(r):
        for jl in range(P):
            m[ri, jl, :] = (P * ri + jl <= il).astype(np.float32)
    return m


def build_nc(s=S, b=B, tchunk=512, mm_dtype=mybir.dt.bfloat16):
    t = b * s
    tchunk = min(tchunk, t)
    nch = t // tchunk             # phase-1 token chunks
    ich = s // tchunk             # attention i-chunks per batch
    r_mask = tchunk // P          # diagonal-crossing tiles per i-chunk
    ntt = t // P                  # token tiles

    FR = mm_dtype

    def mm(ap):
        return ap

    nc = bacc.Bacc("TRN2", target_bir_lowering=False, debug=False)

    xt = nc.declare_dram_parameter("xt", [H, t], FR, isOutput=False)
    wqt = nc.declare_dram_parameter("wqt", [H, M], FR, isOutput=False)
    wkt = nc.declare_dram_parameter("wkt", [H, M], FR, isOutput=False)
    wvt = nc.declare_dram_parameter("wvt", [H, M], FR, isOutput=False)
    wot = nc.declare_dram_parameter("wot", [M, H], FR, isOutput=False)
    cost = nc.declare_dram_parameter("cost", [HD, t], F32, isOutput=False)
    sint = nc.declare_dram_parameter("sint", [HD, t], F32, isOutput=False)
    masks = nc.declare_dram_parameter("masks", [r_mask, P, tchunk], FR,
                                      isOutput=False)
    out = nc.declare_dram_parameter("out", [t, H], FR, isOutput=True)

    with tile.TileContext(nc) as tc, ExitStack() as ctx:
        persist = ctx.enter_context(tc.tile_pool(name="persist", bufs=1))

        # persistent activations
        qr = [persist.tile([P, t], FR, tag=f"qr{h}", name=f"qr{h}") for h in range(HPC)]
        kr = [persist.tile([P, t], FR, tag=f"kr{h}", name=f"kr{h}") for h in range(HPC)]
        vv = persist.tile([P, ntt, M], FR, tag="vv")   # v[tt*128+p, d]
        ones_s = persist.tile([P, P], FR, tag="ones")
        nc.vector.memset(ones_s[:], 1.0)
        # allocated up-front (fresh SBUF -> no reuse waits on their DMAs);
        # loads issued after phase 1 so they don't delay the first matmuls
        mask_s = persist.tile([P, r_mask, tchunk], FR, tag="masks")
        wo_s = persist.tile([P, HPC, H], FR, tag="wo")
        ev_pool = ctx.enter_context(tc.tile_pool(name="evp", bufs=8))
        # whole-kernel 2-bank PSUM tiles: phase-1 q/k accumulator pairs and
        # attention score tiles rotate through the same two slots (A, B) --
        # no pool-handoff barrier on the critical QK path
        ab_pool = ctx.enter_context(tc.tile_pool(name="ab", bufs=1, space="PSUM"))

        # ---------------- phase 1: projections + rope -----------------
        with (
            tc.tile_pool(name="csin", bufs=2) as csin_pool,
            tc.tile_pool(name="xtp", bufs=3) as xt_pool,
            tc.tile_pool(name="rtmp", bufs=3) as rtmp_pool,
            tc.tile_pool(name="wts", bufs=1) as wts_pool,
            tc.tile_pool(name="p1v", bufs=1, space="PSUM") as p1v,
        ):
            wq_s = wts_pool.tile([P, NKT, M], FR, tag="wq")
            wk_s = wts_pool.tile([P, NKT, M], FR, tag="wk")
            wv_s = wts_pool.tile([P, NKT, M], FR, tag="wv")
            KG = 4                       # k-tiles per DMA
            for c in range(nch):
                tsl = slice(c * tchunk, (c + 1) * tchunk)
                cos_t = csin_pool.tile([P, tchunk], F32, tag="cos")
                sin_t = csin_pool.tile([P, tchunk], F32, tag="sin")

                # kt-outer: each X.T k-tile feeds all 8 accumulators, then dies
                q_ps = ab_pool.tile([P, HPC, 512], F32, tag="A", name=f"qps_{c}")
                k_ps = ab_pool.tile([P, HPC, 512], F32, tag="B", name=f"kps_{c}")
                qk_ps = [q_ps[:, 0, :tchunk], q_ps[:, 1, :tchunk],
                         k_ps[:, 0, :tchunk], k_ps[:, 1, :tchunk]]
                nvp = tchunk // P
                v_ps = [p1v.tile([P, M], F32, tag=f"p1v{i}",
                                 name=f"p1v{i}_{c}") for i in range(nvp)]
                for g in range(NKT // KG):
                    gsl = slice(g * KG * P, (g + 1) * KG * P)
                    if c == 0:
                        # weight loads on the (otherwise idle) gpsimd SWDGE
                        # queue: they issue in parallel with SP's x loads
                        for w_s, wsrc in ((wq_s, wqt), (wk_s, wkt),
                                          (wv_s, wvt)):
                            nc.gpsimd.dma_start(
                                out=w_s[:, g * KG:(g + 1) * KG, :],
                                in_=wsrc[gsl, :].rearrange(
                                    "(k p) m -> p k m", p=P))
                    xt4 = xt_pool.tile([P, KG, tchunk], FR, tag="xt")
                    nc.sync.dma_start(
                        out=xt4[:],
                        in_=xt[gsl, tsl].rearrange("(k p) t -> p k t", p=P))
                    for kk in range(KG):
                        kt = g * KG + kk
                        fl = dict(start=(kt == 0), stop=(kt == NKT - 1))
                        for wi, w_s in enumerate((wq_s, wk_s)):
                            for h in range(HPC):
                                msl = slice(h * P, (h + 1) * P)
                                nc.tensor.matmul(qk_ps[wi * HPC + h][:],
                                                 mm(w_s[:, kt, msl]),
                                                 mm(xt4[:, kk, :]), **fl)
                        for ts_ in range(nvp):
                            ssl = slice(ts_ * P, (ts_ + 1) * P)
                            nc.tensor.matmul(v_ps[ts_][:],
                                             mm(xt4[:, kk, ssl]),
                                             mm(wv_s[:, kt, :]), **fl)

                nc.gpsimd.dma_start(out=cos_t[:], in_=cost[:, tsl])
                nc.gpsimd.dma_start(out=sin_t[:], in_=sint[:, tsl])

                # rope eviction: dest = ps*cos + shuffle(ps)*sin_eff
                for wi, dest in ((0, qr), (1, kr)):
                    for h in range(HPC):
                        ps = qk_ps[wi * HPC + h]
                        shuf = rtmp_pool.tile([P, tchunk], F32, tag="shuf")
                        dst = dest[h][:, tsl]
                        nc.vector.stream_shuffle(out=shuf[:], in_=ps,
                                                 mask=SWAP_MASK)
                        nc.vector.tensor_mul(out=dst, in0=ps, in1=cos_t[:])
                        nc.vector.tensor_mul(out=shuf[:], in0=shuf[:], in1=sin_t[:])
                        nc.vector.tensor_add(out=dst, in0=dst, in1=shuf[:])

                # v eviction: token-major
                for ts_ in range(nvp):
                    nc.vector.tensor_copy(out=vv[:, c * nvp + ts_, :],
                                          in_=v_ps[ts_][:])

        nc.sync.dma_start(out=mask_s[:], in_=masks.rearrange("r p n -> p r n"))
        nc.sync.dma_start(out=wo_s[:],
                          in_=wot.rearrange("(mt p) o -> p mt o", p=P))

        # -------- phase 2+3: attention with interleaved output proj -------
        # Software-pipelined: QK for tile jt+1 issues before colsum/AV of jt,
        # and both heads' exp runs as ONE wide ACT op over a 2-bank PSUM
        # tile, so ACT latency never blocks the PE stream.
        with (
            tc.tile_pool(name="outp", bufs=1) as out_pool,
            tc.tile_pool(name="exps", bufs=8) as exps_pool,
            tc.tile_pool(name="rcp", bufs=2) as rcp_pool,
            tc.tile_pool(name="p2cs", bufs=1, space="PSUM") as p2cs,
            tc.tile_pool(name="p2av", bufs=1, space="PSUM") as p2av,
        ):
            outT = [out_pool.tile([P, t], FR, tag=f"outT{h}", name=f"outT{h}")
                    for h in range(HPC)]

            def drain_one(pend):
                (pes, plo, pw, pfl, pjt, ctx_) = pend.pop(0)
                (bb_, cs_l, av_l, isl_, c_) = ctx_
                for h in range(HPC):
                    nc.tensor.matmul(cs_l[h][:, plo:], mm(ones_s[:]),
                                     mm(pes[:, h, :pw]), **pfl)
                    nc.tensor.matmul(av_l[h][:, plo:],
                                     mm(vv[:, bb_ * (s // P) + pjt,
                                           h * P:(h + 1) * P]),
                                     mm(pes[:, h, :pw]), **pfl)
                if not pfl["stop"]:
                    return
                # chunk epilogue: normalize + output projection
                for h in range(HPC):
                    rcp = rcp_pool.tile([P, tchunk], F32, tag="rcp",
                                        name=f"rcp{h}_{bb_}_{c_}")
                    nc.vector.reciprocal_approx_fast(out=rcp[:], in_=cs_l[h][:])
                    nc.vector.tensor_mul(out=outT[h][:, isl_], in0=av_l[h][:],
                                         in1=rcp[:])
                wo_pools = [p2cs, p2cs, p2av, p2av]
                wo_tags = ["cs0", "cs1", "av0", "av1"]
                wi_ = 0
                for tt_ in range(tchunk // P):
                    tt0 = isl_.start + tt_ * P
                    ttsl = slice(tt0, tt0 + P)
                    for oc in range(H // 512):
                        osl = slice(oc * 512, (oc + 1) * 512)
                        ps = wo_pools[wi_ % 4].tile(
                            [P, 512], F32, tag=wo_tags[wi_ % 4],
                            name=f"wo_{tt0}_{oc}")
                        wi_ += 1
                        for h in range(HPC):
                            nc.tensor.matmul(ps[:],
                                             mm(outT[h][:, ttsl]),
                                             mm(wo_s[:, h, osl]),
                                             start=(h == 0),
                                             stop=(h == HPC - 1))
                        ev = ev_pool.tile([P, 512], FR, tag="ev",
                                          name=f"ev_{tt0}_{oc}")
                        nc.vector.tensor_copy(out=ev[:], in_=ps[:])
                        nc.sync.dma_start(out=out[ttsl, osl], in_=ev[:])

            pend = []
            for bb in range(b):
                for c in range(ich):
                    isl = slice(bb * s + c * tchunk, bb * s + (c + 1) * tchunk)
                    njt = r_mask * (c + 1)   # visible j-tiles
                    cs_ps = [p2cs.tile([P, tchunk], F32, tag=f"cs{h}",
                                       name=f"cs{h}_{bb}_{c}") for h in range(HPC)]
                    av_ps = [p2av.tile([P, tchunk], F32, tag=f"av{h}",
                                       name=f"av{h}_{bb}_{c}") for h in range(HPC)]
                    cctx = (bb, cs_ps, av_ps, isl, c)
                    for jt in range(njt):
                        jsl = slice(bb * s + jt * P, bb * s + (jt + 1) * P)
                        ri = jt - r_mask * c
                        lo = max(ri, 0) * P
                        w = tchunk - lo
                        csl = slice(isl.start + lo, isl.stop)
                        fl = dict(start=(jt == 0), stop=(jt == njt - 1))
                        sc = ab_pool.tile([P, HPC, 512], F32,
                                          tag=("A", "B")[jt % 2],
                                          name=f"sc_{bb}_{c}_{jt}")
                        for h in range(HPC):
                            nc.tensor.matmul(sc[:, h, :w], mm(kr[h][:, jsl]),
                                             mm(qr[h][:, csl]),
                                             start=True, stop=True)
                        es = exps_pool.tile([P, HPC, tchunk], FR, tag="es",
                                            name=f"es_{bb}_{c}_{jt}")
                        nc.scalar.activation(out=es[:, :, :w], in_=sc[:, :, :w],
                                             func=mybir.ActivationFunctionType.Exp,
                                             scale=float(SCALE))
                        if ri >= 0:  # diagonal-crossing tile
                            mb = mask_s[:, ri, lo:].unsqueeze(1).broadcast_to(
                                [P, HPC, w])
                            nc.vector.tensor_mul(out=es[:, :, :w],
                                                 in0=es[:, :, :w], in1=mb)
                        pend.append((es, lo, w, fl, jt, cctx))
                        if len(pend) > 2:
                            drain_one(pend)
            while pend:
                drain_one(pend)

    nc.compile()
    return nc


def make_in_maps(hidden_states, cos, sin, Wq, Wk, Wv, Wo, s=S, b=B, tchunk=512):
    t = b * s
    tchunk = min(tchunk, t)
    hs = np.asarray(hidden_states, np.float32).reshape(t, H)
    xt = np.ascontiguousarray(hs.T)
    cos2 = np.asarray(cos, np.float32).reshape(s, HD)
    sin2 = np.asarray(sin, np.float32).reshape(s, HD)
    cosP = np.ascontiguousarray(np.tile(cos2[:, PERM].T, (1, b)))
    sign = np.where(PERM < 64, -1.0, 1.0).astype(np.float32)[:, None]
    sinP = np.ascontiguousarray(np.tile(sin2[:, PERM].T * sign, (1, b)))
    masks_bf = build_masks(tchunk).astype(BF16)
    xt_bf = xt.astype(BF16)
    Wq, Wk, Wv, Wo = (np.asarray(w, np.float32) for w in (Wq, Wk, Wv, Wo))

    in_maps = []
    for c in range(NCORES):
        rows = np.concatenate([(HPC * c + hh) * HD + PERM for hh in range(HPC)])
        sl = slice(c * M, (c + 1) * M)
        in_maps.append({
            "xt": xt_bf,
            "wqt": np.ascontiguousarray(Wq[rows, :].T).astype(BF16),
            "wkt": np.ascontiguousarray(Wk[rows, :].T).astype(BF16),
            "wvt": np.ascontiguousarray(Wv[sl, :].T).astype(BF16),
            "wot": np.ascontiguousarray(Wo[:, sl].T).astype(BF16),
            "cost": cosP,
            "sint": sinP,
            "masks": masks_bf,
        })
    return in_maps


_CACHED_NC = None
_LAST_RESULTS = None


def kernel(hidden_states, cos, sin, Wq, Wk, Wv, Wo):
    global _CACHED_NC, _LAST_RESULTS
    in_maps = make_in_maps(hidden_states, cos, sin, Wq, Wk, Wv, Wo)
    if _CACHED_NC is None:
        _CACHED_NC = build_nc()
    res = run_bass_kernel_spmd(_CACHED_NC, in_maps, core_ids=list(range(NCORES)))
    _LAST_RESULTS = res
    acc = np.zeros((B * S, H), np.float32)
    for r in res.results:
        acc += r["out"].astype(np.float32)
    return acc.reshape(B, S, H)



# revision 6
# speedup vs baseline: 1.0521x; 1.0521x over previous
"""Trainium2 Bass kernel for nn_Attention_3126736192307.

Causal multi-head attention with RoPE: B=2, S=2048, H=2048, 16 heads x 128.

Sharding (tensor parallel over heads, 8 cores, 2 heads each):
  - Wq/Wk/Wv column-split (per-head), Wo row-split; each core computes a
    partial [B*S, H] output; the host sums the 8 partials (row-parallel
    unshard) - no on-device collectives needed.

Per-core dataflow (all matmuls transpose-free by construction):
  - Host pre-transposes: X.T [H, T], WqT/WkT [H, 256] (head-dim permuted so
    RoPE's rotate_half becomes an intra-quadrant stream_shuffle), WvT [H, 256],
    WoT [256, H], cos/sin [128, T] feature-major (sin sign-folded).
  - Phase 1: q,k feature-major [128, T] per head + RoPE (DVE); v token-major.
  - Phase 2 per (b, h, i-chunk): scores.T [j,i] = k.T (lhsT) @ q.T; exp on
    ScalarE (no max subtraction - scores are ~N(0,1) after the 1/sqrt(hd)
    scale); causal block skipping + 0/1 mask multiply on diagonal-crossing
    tiles; column sums via ones-matmul on TensorE; AV accumulation in PSUM;
    normalization folded into the PSUM->SBUF eviction.
  - Phase 3: out.T (lhsT) @ WoT -> partial [T, H], streamed to DRAM.

Matmuls run in bf16 (1 PE cycle/row; fp32 is 4x, and fp32r's fused
weight-load encoding can't carry the 2 semaphore waits Tile emits).
"""

import os
import sys

for _p in ("/opt/trn_rl_repo", "/root/.axon_site/_ro/trn_rl_repo"):
    if os.path.isdir(_p) and _p not in sys.path:
        sys.path.append(_p)

from contextlib import ExitStack

import ml_dtypes
import numpy as np

import concourse.bass as bass
import concourse.bacc as bacc
import concourse.tile as tile
from concourse import mybir
from concourse.bass_utils import run_bass_kernel_spmd

B, S, H, NH = 2, 2048, 2048, 16
HD = 128
NCORES = 8
HPC = NH // NCORES            # heads per core = 2
M = HPC * HD                  # 256 output channels per core
SCALE = HD ** -0.5
P = 128                       # partitions
NKT = H // P                  # 16 contraction tiles for projections

F32 = mybir.dt.float32

# head-dim permutation: interleave halves at 16 granularity so the RoPE
# partner (d <-> d+64) sits 16 partitions away inside one 32-part quadrant
PERM = np.concatenate([np.arange(16 * m, 16 * m + 16) + (64 if odd else 0)
                       for m in range(4) for odd in (0, 1)])
SWAP_MASK = [i ^ 16 for i in range(32)]


BF16 = ml_dtypes.bfloat16


def build_masks(tchunk):
    """0/1 keep-masks for the R diagonal-crossing j-tiles of each i-chunk."""
    r = tchunk // P
    m = np.zeros((r, P, tchunk), np.float32)
    il = np.arange(tchunk)
    for ri in range(r):
        for jl in range(P):
            m[ri, jl, :] = (P * ri + jl <= il).astype(np.float32)
    return m


def build_nc(s=S, b=B, tchunk=512, mm_dtype=mybir.dt.bfloat16):
    t = b * s
    tchunk = min(tchunk, t)
    nch = t // tchunk             # phase-1 token chunks
    ich = s // tchunk             # attention i-chunks per batch
    r_mask = tchunk // P          # diagonal-crossing tiles per i-chunk
    ntt = t // P                  # token tiles

    FR = mm_dtype

    def mm(ap):
        return ap

    nc = bacc.Bacc("TRN2", target_bir_lowering=False, debug=False)

    xt = nc.declare_dram_parameter("xt", [H, t], FR, isOutput=False)
    wqt = nc.declare_dram_parameter("wqt", [H, M], FR, isOutput=False)
    wkt = nc.declare_dram_parameter("wkt", [H, M], FR, isOutput=False)
    wvt = nc.declare_dram_parameter("wvt", [H, M], FR, isOutput=False)
    wot = nc.declare_dram_parameter("wot", [M, H], FR, isOutput=False)
    cost = nc.declare_dram_parameter("cost", [HD, t], FR, isOutput=False)
    sint = nc.declare_dram_parameter("sint", [HD, t], FR, isOutput=False)
    masks = nc.declare_dram_parameter("masks", [r_mask, P, tchunk], FR,
                                      isOutput=False)
    out = nc.declare_dram_parameter("out", [t, H], FR, isOutput=True)

    with tile.TileContext(nc) as tc, ExitStack() as ctx:
        persist = ctx.enter_context(tc.tile_pool(name="persist", bufs=1))

        # persistent activations
        qr = [persist.tile([P, t], FR, tag=f"qr{h}", name=f"qr{h}") for h in range(HPC)]
        kr = [persist.tile([P, t], FR, tag=f"kr{h}", name=f"kr{h}") for h in range(HPC)]
        vv = persist.tile([P, ntt, M], FR, tag="vv")   # v[tt*128+p, d]
        ones_s = persist.tile([P, P], FR, tag="ones")
        nc.vector.memset(ones_s[:], 1.0)
        # allocated up-front (fresh SBUF -> no reuse waits on their DMAs)
        mask_s = persist.tile([P, r_mask, tchunk], FR, tag="masks")
        wo_s = persist.tile([P, HPC, H], FR, tag="wo")
        cos_sb = persist.tile([P, t], FR, tag="cosb")
        sin_sb = persist.tile([P, t], FR, tag="sinb")
        ev_pool = ctx.enter_context(tc.tile_pool(name="evp", bufs=8))
        # whole-kernel 2-bank PSUM tiles: phase-1 q/k accumulator pairs and
        # attention score tiles rotate through the same two slots (A, B) --
        # no pool-handoff barrier on the critical QK path
        ab_pool = ctx.enter_context(tc.tile_pool(name="ab", bufs=1, space="PSUM"))

        # ---------------- phase 1: projections + rope -----------------
        with (
            tc.tile_pool(name="xtp", bufs=6) as xt_pool,
            tc.tile_pool(name="qkt", bufs=2) as qkt_pool,
            tc.tile_pool(name="rtmp", bufs=3) as rtmp_pool,
            tc.tile_pool(name="wts", bufs=1) as wts_pool,
            tc.tile_pool(name="p1v", bufs=1, space="PSUM") as p1v,
        ):
            wq_s = wts_pool.tile([P, NKT, M], FR, tag="wq")
            wk_s = wts_pool.tile([P, NKT, M], FR, tag="wk")
            wv_s = wts_pool.tile([P, NKT, M], FR, tag="wv")
            KG = 4                       # k-tiles per DMA
            # all long-lived loads upfront on the gpsimd SWDGE queue, in
            # order of first use: weights (c0 matmuls), cos/sin (c0
            # eviction), masks/wo (phase 2)
            for g in range(NKT // KG):
                gsl = slice(g * KG * P, (g + 1) * KG * P)
                for w_s, wsrc in ((wq_s, wqt), (wk_s, wkt), (wv_s, wvt)):
                    nc.gpsimd.dma_start(
                        out=w_s[:, g * KG:(g + 1) * KG, :],
                        in_=wsrc[gsl, :].rearrange("(k p) m -> p k m", p=P))
            nc.gpsimd.dma_start(out=cos_sb[:], in_=cost[:, :])
            nc.gpsimd.dma_start(out=sin_sb[:], in_=sint[:, :])
            nc.gpsimd.dma_start(out=mask_s[:],
                                in_=masks.rearrange("r p n -> p r n"))
            nc.gpsimd.dma_start(out=wo_s[:],
                                in_=wot.rearrange("(mt p) o -> p mt o", p=P))

            for c in range(nch):
                tsl = slice(c * tchunk, (c + 1) * tchunk)

                # kt-outer: each X.T k-tile feeds all 8 accumulators, then dies
                q_ps = ab_pool.tile([P, HPC, 512], F32, tag="A", name=f"qps_{c}")
                k_ps = ab_pool.tile([P, HPC, 512], F32, tag="B", name=f"kps_{c}")
                qk_ps = [q_ps[:, 0, :tchunk], q_ps[:, 1, :tchunk],
                         k_ps[:, 0, :tchunk], k_ps[:, 1, :tchunk]]
                nvp = tchunk // P
                v_ps = [p1v.tile([P, M], F32, tag=f"p1v{i}",
                                 name=f"p1v{i}_{c}") for i in range(nvp)]
                for g in range(NKT // KG):
                    gsl = slice(g * KG * P, (g + 1) * KG * P)
                    xt4 = xt_pool.tile([P, KG, tchunk], FR, tag="xt")
                    nc.sync.dma_start(
                        out=xt4[:],
                        in_=xt[gsl, tsl].rearrange("(k p) t -> p k t", p=P))
                    for kk in range(KG):
                        kt = g * KG + kk
                        fl = dict(start=(kt == 0), stop=(kt == NKT - 1))
                        for wi, w_s in enumerate((wq_s, wk_s)):
                            for h in range(HPC):
                                msl = slice(h * P, (h + 1) * P)
                                nc.tensor.matmul(qk_ps[wi * HPC + h][:],
                                                 mm(w_s[:, kt, msl]),
                                                 mm(xt4[:, kk, :]), **fl)
                        for ts_ in range(nvp):
                            ssl = slice(ts_ * P, (ts_ + 1) * P)
                            nc.tensor.matmul(v_ps[ts_][:],
                                             mm(xt4[:, kk, ssl]),
                                             mm(wv_s[:, kt, :]), **fl)

                # early PSUM release: ScalarE (idle in phase 1) copies the
                # accumulators to SBUF bf16; the A/B banks free after ~1us
                # instead of after the full DVE rope chain, so the next
                # chunk's matmuls start immediately
                qkt = qkt_pool.tile([P, 2, HPC, tchunk], FR, tag="qkt",
                                    name=f"qkt_{c}")
                nc.scalar.copy(out=qkt[:, 0], in_=q_ps[:, :, :tchunk])
                nc.scalar.copy(out=qkt[:, 1], in_=k_ps[:, :, :tchunk])
                # v eviction: token-major (also ScalarE, frees p1v banks)
                for ts_ in range(nvp):
                    nc.scalar.copy(out=vv[:, c * nvp + ts_, :],
                                   in_=v_ps[ts_][:])

                # rope on DVE, all-bf16 (2x DVE throughput):
                # dest = qk*cos + shuffle(qk)*sin_eff
                for wi, dest in ((0, qr), (1, kr)):
                    for h in range(HPC):
                        src = qkt[:, wi, h, :]
                        shuf = rtmp_pool.tile([P, tchunk], FR, tag="shuf")
                        dst = dest[h][:, tsl]
                        nc.vector.stream_shuffle(out=shuf[:], in_=src,
                                                 mask=SWAP_MASK)
                        nc.vector.tensor_mul(out=dst, in0=src,
                                             in1=cos_sb[:, tsl])
                        nc.vector.tensor_mul(out=shuf[:], in0=shuf[:],
                                             in1=sin_sb[:, tsl])
                        nc.vector.tensor_add(out=dst, in0=dst, in1=shuf[:])

        # -------- phase 2+3: attention with interleaved output proj -------
        # Software-pipelined: QK for tile jt+1 issues before colsum/AV of jt,
        # and both heads' exp runs as ONE wide ACT op over a 2-bank PSUM
        # tile, so ACT latency never blocks the PE stream.
        with (
            tc.tile_pool(name="outp", bufs=1) as out_pool,
            tc.tile_pool(name="exps", bufs=8) as exps_pool,
            tc.tile_pool(name="rcp", bufs=2) as rcp_pool,
            tc.tile_pool(name="p2cs", bufs=1, space="PSUM") as p2cs,
            tc.tile_pool(name="p2av", bufs=1, space="PSUM") as p2av,
        ):
            outT = [out_pool.tile([P, t], FR, tag=f"outT{h}", name=f"outT{h}")
                    for h in range(HPC)]

            def drain_one(pend):
                (pes, plo, pw, pfl, pjt, ctx_) = pend.pop(0)
                (bb_, cs_l, av_l, isl_, c_) = ctx_
                for h in range(HPC):
                    nc.tensor.matmul(cs_l[h][:, plo:], mm(ones_s[:]),
                                     mm(pes[:, h, :pw]), **pfl)
                    nc.tensor.matmul(av_l[h][:, plo:],
                                     mm(vv[:, bb_ * (s // P) + pjt,
                                           h * P:(h + 1) * P]),
                                     mm(pes[:, h, :pw]), **pfl)
                if not pfl["stop"]:
                    return
                # chunk epilogue: normalize + output projection
                for h in range(HPC):
                    rcp = rcp_pool.tile([P, tchunk], F32, tag="rcp",
                                        name=f"rcp{h}_{bb_}_{c_}")
                    nc.vector.reciprocal_approx_fast(out=rcp[:], in_=cs_l[h][:])
                    nc.vector.tensor_mul(out=outT[h][:, isl_], in0=av_l[h][:],
                                         in1=rcp[:])
                wo_pools = [p2cs, p2cs, p2av, p2av]
                wo_tags = ["cs0", "cs1", "av0", "av1"]
                wi_ = 0
                for tt_ in range(tchunk // P):
                    tt0 = isl_.start + tt_ * P
                    ttsl = slice(tt0, tt0 + P)
                    for oc in range(H // 512):
                        osl = slice(oc * 512, (oc + 1) * 512)
                        ps = wo_pools[wi_ % 4].tile(
                            [P, 512], F32, tag=wo_tags[wi_ % 4],
                            name=f"wo_{tt0}_{oc}")
                        wi_ += 1
                        for h in range(HPC):
                            nc.tensor.matmul(ps[:],
                                             mm(outT[h][:, ttsl]),
                                             mm(wo_s[:, h, osl]),
                                             start=(h == 0),
                                             stop=(h == HPC - 1))
                        ev = ev_pool.tile([P, 512], FR, tag="ev",
                                          name=f"ev_{tt0}_{oc}")
                        # alternate the PSUM->SBUF eviction between DVE and
                        # ScalarE so neither engine eats the whole 88us
                        if wi_ % 2:
                            nc.scalar.copy(out=ev[:], in_=ps[:])
                        else:
                            nc.vector.tensor_copy(out=ev[:], in_=ps[:])
                        nc.sync.dma_start(out=out[ttsl, osl], in_=ev[:])

            pend = []
            for bb in range(b):
                for c in range(ich):
                    isl = slice(bb * s + c * tchunk, bb * s + (c + 1) * tchunk)
                    njt = r_mask * (c + 1)   # visible j-tiles
                    cs_ps = [p2cs.tile([P, tchunk], F32, tag=f"cs{h}",
                                       name=f"cs{h}_{bb}_{c}") for h in range(HPC)]
                    av_ps = [p2av.tile([P, tchunk], F32, tag=f"av{h}",
                                       name=f"av{h}_{bb}_{c}") for h in range(HPC)]
                    cctx = (bb, cs_ps, av_ps, isl, c)
                    for jt in range(njt):
                        jsl = slice(bb * s + jt * P, bb * s + (jt + 1) * P)
                        ri = jt - r_mask * c
                        lo = max(ri, 0) * P
                        w = tchunk - lo
                        csl = slice(isl.start + lo, isl.stop)
                        fl = dict(start=(jt == 0), stop=(jt == njt - 1))
                        sc = ab_pool.tile([P, HPC, 512], F32,
                                          tag=("A", "B")[jt % 2],
                                          name=f"sc_{bb}_{c}_{jt}")
                        for h in range(HPC):
                            nc.tensor.matmul(sc[:, h, :w], mm(kr[h][:, jsl]),
                                             mm(qr[h][:, csl]),
                                             start=True, stop=True)
                        es = exps_pool.tile([P, HPC, tchunk], FR, tag="es",
                                            name=f"es_{bb}_{c}_{jt}")
                        nc.scalar.activation(out=es[:, :, :w], in_=sc[:, :, :w],
                                             func=mybir.ActivationFunctionType.Exp,
                                             scale=float(SCALE))
                        if ri >= 0:  # diagonal-crossing tile
                            mb = mask_s[:, ri, lo:].unsqueeze(1).broadcast_to(
                                [P, HPC, w])
                            nc.vector.tensor_mul(out=es[:, :, :w],
                                                 in0=es[:, :, :w], in1=mb)
                        pend.append((es, lo, w, fl, jt, cctx))
                        if len(pend) > 2:
                            drain_one(pend)
            while pend:
                drain_one(pend)

    nc.compile()
    return nc


def make_in_maps(hidden_states, cos, sin, Wq, Wk, Wv, Wo, s=S, b=B, tchunk=512):
    t = b * s
    tchunk = min(tchunk, t)
    hs = np.asarray(hidden_states, np.float32).reshape(t, H)
    xt = np.ascontiguousarray(hs.T)
    cos2 = np.asarray(cos, np.float32).reshape(s, HD)
    sin2 = np.asarray(sin, np.float32).reshape(s, HD)
    cosP = np.ascontiguousarray(np.tile(cos2[:, PERM].T, (1, b))).astype(BF16)
    sign = np.where(PERM < 64, -1.0, 1.0).astype(np.float32)[:, None]
    sinP = np.ascontiguousarray(
        np.tile(sin2[:, PERM].T * sign, (1, b))).astype(BF16)
    masks_bf = build_masks(tchunk).astype(BF16)
    xt_bf = xt.astype(BF16)
    Wq, Wk, Wv, Wo = (np.asarray(w, np.float32) for w in (Wq, Wk, Wv, Wo))

    in_maps = []
    for c in range(NCORES):
        rows = np.concatenate([(HPC * c + hh) * HD + PERM for hh in range(HPC)])
        sl = slice(c * M, (c + 1) * M)
        in_maps.append({
            "xt": xt_bf,
            "wqt": np.ascontiguousarray(Wq[rows, :].T).astype(BF16),
            "wkt": np.ascontiguousarray(Wk[rows, :].T).astype(BF16),
            "wvt": np.ascontiguousarray(Wv[sl, :].T).astype(BF16),
            "wot": np.ascontiguousarray(Wo[:, sl].T).astype(BF16),
            "cost": cosP,
            "sint": sinP,
            "masks": masks_bf,
        })
    return in_maps


_CACHED_NC = None
_LAST_RESULTS = None


def kernel(hidden_states, cos, sin, Wq, Wk, Wv, Wo):
    global _CACHED_NC, _LAST_RESULTS
    in_maps = make_in_maps(hidden_states, cos, sin, Wq, Wk, Wv, Wo)
    if _CACHED_NC is None:
        _CACHED_NC = build_nc()
    res = run_bass_kernel_spmd(_CACHED_NC, in_maps, core_ids=list(range(NCORES)))
    _LAST_RESULTS = res
    acc = np.zeros((B * S, H), np.float32)
    for r in res.results:
        acc += r["out"].astype(np.float32)
    return acc.reshape(B, S, H)



# revision 10
# speedup vs baseline: 1.0759x; 1.0226x over previous
"""Trainium2 Bass kernel for nn_Attention_3126736192307.

Causal multi-head attention with RoPE: B=2, S=2048, H=2048, 16 heads x 128.

Sharding (tensor parallel over heads, 8 cores, 2 heads each):
  - Wq/Wk/Wv column-split (per-head), Wo row-split; each core computes a
    partial [B*S, H] output; the host sums the 8 partials (row-parallel
    unshard) - no on-device collectives needed.

Per-core dataflow (all matmuls transpose-free by construction):
  - Host pre-transposes: X.T [H, T], WqT/WkT [H, 256] (head-dim permuted so
    RoPE's rotate_half becomes an intra-quadrant stream_shuffle), WvT [H, 256],
    WoT [256, H], cos/sin [128, T] feature-major (sin sign-folded).
  - Phase 1: q,k feature-major [128, T] per head + RoPE (DVE); v token-major.
  - Phase 2 per (b, h, i-chunk): scores.T [j,i] = k.T (lhsT) @ q.T; exp on
    ScalarE (no max subtraction - scores are ~N(0,1) after the 1/sqrt(hd)
    scale); causal block skipping + 0/1 mask multiply on diagonal-crossing
    tiles; column sums via ones-matmul on TensorE; AV accumulation in PSUM;
    normalization folded into the PSUM->SBUF eviction.
  - Phase 3: out.T (lhsT) @ WoT -> partial [T, H], streamed to DRAM.

Matmuls run in bf16 (1 PE cycle/row; fp32 is 4x, and fp32r's fused
weight-load encoding can't carry the 2 semaphore waits Tile emits).
"""

import os
import sys

for _p in ("/opt/trn_rl_repo", "/root/.axon_site/_ro/trn_rl_repo"):
    if os.path.isdir(_p) and _p not in sys.path:
        sys.path.append(_p)

from contextlib import ExitStack

import ml_dtypes
import numpy as np

import concourse.bass as bass
import concourse.bacc as bacc
import concourse.tile as tile
from concourse import mybir
from concourse.bass_utils import run_bass_kernel_spmd

B, S, H, NH = 2, 2048, 2048, 16
HD = 128
NCORES = 8
HPC = NH // NCORES            # heads per core = 2
M = HPC * HD                  # 256 output channels per core
SCALE = HD ** -0.5
P = 128                       # partitions
NKT = H // P                  # 16 contraction tiles for projections

F32 = mybir.dt.float32

# head-dim permutation: interleave halves at 16 granularity so the RoPE
# partner (d <-> d+64) sits 16 partitions away inside one 32-part quadrant
PERM = np.concatenate([np.arange(16 * m, 16 * m + 16) + (64 if odd else 0)
                       for m in range(4) for odd in (0, 1)])
SWAP_MASK = [i ^ 16 for i in range(32)]


BF16 = ml_dtypes.bfloat16


def build_masks(tchunk):
    """0/1 keep-masks for the R diagonal-crossing j-tiles of each i-chunk."""
    r = tchunk // P
    m = np.zeros((r, P, tchunk), np.float32)
    il = np.arange(tchunk)
    for ri in range(r):
        for jl in range(P):
            m[ri, jl, :] = (P * ri + jl <= il).astype(np.float32)
    return m


def build_nc(s=S, b=B, tchunk=512, mm_dtype=mybir.dt.bfloat16):
    t = b * s
    tchunk = min(tchunk, t)
    nch = t // tchunk             # phase-1 token chunks
    ich = s // tchunk             # attention i-chunks per batch
    r_mask = tchunk // P          # diagonal-crossing tiles per i-chunk
    ntt = t // P                  # token tiles

    FR = mm_dtype

    def mm(ap):
        return ap

    nc = bacc.Bacc("TRN2", target_bir_lowering=False, debug=False)

    xt = nc.declare_dram_parameter("xt", [H, t], FR, isOutput=False)
    wqt = nc.declare_dram_parameter("wqt", [H, M], FR, isOutput=False)
    wkt = nc.declare_dram_parameter("wkt", [H, M], FR, isOutput=False)
    wvt = nc.declare_dram_parameter("wvt", [H, M], FR, isOutput=False)
    wot = nc.declare_dram_parameter("wot", [M, H], FR, isOutput=False)
    cost = nc.declare_dram_parameter("cost", [HD, t], FR, isOutput=False)
    sint = nc.declare_dram_parameter("sint", [HD, t], FR, isOutput=False)
    masks = nc.declare_dram_parameter("masks", [r_mask, P, tchunk], FR,
                                      isOutput=False)
    out = nc.declare_dram_parameter("out", [t, H], FR, isOutput=True)

    with tile.TileContext(nc) as tc, ExitStack() as ctx:
        persist = ctx.enter_context(tc.tile_pool(name="persist", bufs=1))

        # persistent activations
        qr = [persist.tile([P, t], FR, tag=f"qr{h}", name=f"qr{h}") for h in range(HPC)]
        kr = [persist.tile([P, t], FR, tag=f"kr{h}", name=f"kr{h}") for h in range(HPC)]
        vv = persist.tile([P, ntt, M], FR, tag="vv")   # v[tt*128+p, d]
        ones_s = persist.tile([P, P], FR, tag="ones")
        nc.vector.memset(ones_s[:], 1.0)
        # allocated up-front (fresh SBUF -> no reuse waits on their DMAs)
        mask_s = persist.tile([P, r_mask, tchunk], FR, tag="masks")
        wo_s = persist.tile([P, HPC, H], FR, tag="wo")
        cos_sb = persist.tile([P, t], FR, tag="cosb")
        sin_sb = persist.tile([P, t], FR, tag="sinb")
        ev_pool = ctx.enter_context(tc.tile_pool(name="evp", bufs=8))
        # whole-kernel 2-bank PSUM tiles: phase-1 q/k accumulator pairs and
        # attention score tiles rotate through the same two slots (A, B) --
        # no pool-handoff barrier on the critical QK path
        ab_pool = ctx.enter_context(tc.tile_pool(name="ab", bufs=1, space="PSUM"))

        # ---------------- phase 1: projections + rope -----------------
        with (
            tc.tile_pool(name="xtp", bufs=6) as xt_pool,
            tc.tile_pool(name="qkt", bufs=2) as qkt_pool,
            tc.tile_pool(name="rtmp", bufs=3) as rtmp_pool,
            tc.tile_pool(name="wts", bufs=1) as wts_pool,
            tc.tile_pool(name="p1v", bufs=1, space="PSUM") as p1v,
        ):
            wq_s = wts_pool.tile([P, NKT, M], FR, tag="wq")
            wk_s = wts_pool.tile([P, NKT, M], FR, tag="wk")
            wv_s = wts_pool.tile([P, NKT, M], FR, tag="wv")
            KG = 4                       # k-tiles per DMA
            # all long-lived loads upfront on the gpsimd SWDGE queue, in
            # order of first use. The very first loads are kt0-only slivers
            # so the first matmul isn't gated on a multi-us bulk transfer.
            for w_s, wsrc in ((wq_s, wqt), (wk_s, wkt), (wv_s, wvt)):
                nc.gpsimd.dma_start(
                    out=w_s[:, 0:1, :],
                    in_=wsrc[0:P, :].rearrange("(k p) m -> p k m", p=P))
            for g in range(NKT // KG):
                lo_kt = 1 if g == 0 else g * KG
                gsl = slice(lo_kt * P, (g + 1) * KG * P)
                for w_s, wsrc in ((wq_s, wqt), (wk_s, wkt), (wv_s, wvt)):
                    nc.gpsimd.dma_start(
                        out=w_s[:, lo_kt:(g + 1) * KG, :],
                        in_=wsrc[gsl, :].rearrange("(k p) m -> p k m", p=P))
            nc.gpsimd.dma_start(out=cos_sb[:], in_=cost[:, :])
            nc.gpsimd.dma_start(out=sin_sb[:], in_=sint[:, :])
            nc.gpsimd.dma_start(out=mask_s[:],
                                in_=masks.rearrange("r p n -> p r n"))
            nc.gpsimd.dma_start(out=wo_s[:],
                                in_=wot.rearrange("(mt p) o -> p mt o", p=P))

            for c in range(nch):
                tsl = slice(c * tchunk, (c + 1) * tchunk)

                # kt-outer: each X.T k-tile feeds all 8 accumulators, then dies
                q_ps = ab_pool.tile([P, HPC, 512], F32, tag="A", name=f"qps_{c}")
                k_ps = ab_pool.tile([P, HPC, 512], F32, tag="B", name=f"kps_{c}")
                qk_ps = [q_ps[:, 0, :tchunk], q_ps[:, 1, :tchunk],
                         k_ps[:, 0, :tchunk], k_ps[:, 1, :tchunk]]
                nvp = tchunk // P
                v_ps = [p1v.tile([P, M], F32, tag=f"p1v{i}",
                                 name=f"p1v{i}_{c}") for i in range(nvp)]
                for g in range(NKT // KG):
                    gsl = slice(g * KG * P, (g + 1) * KG * P)
                    xt4 = xt_pool.tile([P, KG, tchunk], FR, tag="xt")
                    if c == 0 and g == 0:
                        # kt0 sliver first so the opening matmul isn't gated
                        # on the full 4-ktile transfer
                        nc.sync.dma_start(
                            out=xt4[:, 0:1, :],
                            in_=xt[0:P, tsl].rearrange("(k p) t -> p k t", p=P))
                        nc.sync.dma_start(
                            out=xt4[:, 1:KG, :],
                            in_=xt[P:KG * P, tsl].rearrange(
                                "(k p) t -> p k t", p=P))
                    else:
                        nc.sync.dma_start(
                            out=xt4[:],
                            in_=xt[gsl, tsl].rearrange("(k p) t -> p k t", p=P))
                    for kk in range(KG):
                        kt = g * KG + kk
                        fl = dict(start=(kt == 0), stop=(kt == NKT - 1))
                        for wi, w_s in enumerate((wq_s, wk_s)):
                            for h in range(HPC):
                                msl = slice(h * P, (h + 1) * P)
                                nc.tensor.matmul(qk_ps[wi * HPC + h][:],
                                                 mm(w_s[:, kt, msl]),
                                                 mm(xt4[:, kk, :]), **fl)
                        for ts_ in range(nvp):
                            ssl = slice(ts_ * P, (ts_ + 1) * P)
                            nc.tensor.matmul(v_ps[ts_][:],
                                             mm(xt4[:, kk, ssl]),
                                             mm(wv_s[:, kt, :]), **fl)

                # early PSUM release: ScalarE (idle in phase 1) copies the
                # accumulators to SBUF bf16; the A/B banks free after ~1us
                # instead of after the full DVE rope chain, so the next
                # chunk's matmuls start immediately. v copies go FIRST --
                # phase 2's cs/av pools inherit the p1v banks, so the last
                # chunk's v eviction is on the phase-2 critical path.
                qkt = qkt_pool.tile([P, 2, HPC, tchunk], FR, tag="qkt",
                                    name=f"qkt_{c}")
                for ts_ in range(nvp):
                    nc.scalar.copy(out=vv[:, c * nvp + ts_, :],
                                   in_=v_ps[ts_][:])
                nc.scalar.copy(out=qkt[:, 0], in_=q_ps[:, :, :tchunk])
                nc.scalar.copy(out=qkt[:, 1], in_=k_ps[:, :, :tchunk])

                # rope on DVE, all-bf16 (2x DVE throughput):
                # dest = qk*cos + shuffle(qk)*sin_eff
                for wi, dest in ((0, qr), (1, kr)):
                    for h in range(HPC):
                        src = qkt[:, wi, h, :]
                        shuf = rtmp_pool.tile([P, tchunk], FR, tag="shuf")
                        dst = dest[h][:, tsl]
                        nc.vector.stream_shuffle(out=shuf[:], in_=src,
                                                 mask=SWAP_MASK)
                        nc.vector.tensor_mul(out=dst, in0=src,
                                             in1=cos_sb[:, tsl])
                        nc.vector.tensor_mul(out=shuf[:], in0=shuf[:],
                                             in1=sin_sb[:, tsl])
                        nc.vector.tensor_add(out=dst, in0=dst, in1=shuf[:])

        # -------- phase 2+3: attention with interleaved output proj -------
        # Software-pipelined: QK for tile jt+1 issues before colsum/AV of jt,
        # and both heads' exp runs as ONE wide ACT op over a 2-bank PSUM
        # tile, so ACT latency never blocks the PE stream.
        with (
            tc.tile_pool(name="outp", bufs=1) as out_pool,
            tc.tile_pool(name="exps", bufs=8) as exps_pool,
            tc.tile_pool(name="rcp", bufs=2) as rcp_pool,
            tc.tile_pool(name="p2cs", bufs=1, space="PSUM") as p2cs,
            tc.tile_pool(name="p2av", bufs=1, space="PSUM") as p2av,
        ):
            outT = [out_pool.tile([P, t], FR, tag=f"outT{h}", name=f"outT{h}")
                    for h in range(HPC)]

            def drain_one(pend):
                (pes, plo, pw, pfl, pjt, ctx_) = pend.pop(0)
                (bb_, cs_l, av_l, isl_, c_) = ctx_
                for h in range(HPC):
                    nc.tensor.matmul(cs_l[h][:, plo:], mm(ones_s[:]),
                                     mm(pes[:, h, :pw]), **pfl)
                    nc.tensor.matmul(av_l[h][:, plo:],
                                     mm(vv[:, bb_ * (s // P) + pjt,
                                           h * P:(h + 1) * P]),
                                     mm(pes[:, h, :pw]), **pfl)
                if not pfl["stop"]:
                    return
                # chunk epilogue: normalize + output projection
                for h in range(HPC):
                    rcp = rcp_pool.tile([P, tchunk], F32, tag="rcp",
                                        name=f"rcp{h}_{bb_}_{c_}")
                    nc.vector.reciprocal_approx_fast(out=rcp[:], in_=cs_l[h][:])
                    nc.vector.tensor_mul(out=outT[h][:, isl_], in0=av_l[h][:],
                                         in1=rcp[:])
                wo_pools = [p2cs, p2cs, p2av, p2av]
                wo_tags = ["cs0", "cs1", "av0", "av1"]
                wi_ = 0
                for tt_ in range(tchunk // P):
                    tt0 = isl_.start + tt_ * P
                    ttsl = slice(tt0, tt0 + P)
                    for oc in range(H // 512):
                        osl = slice(oc * 512, (oc + 1) * 512)
                        ps = wo_pools[wi_ % 4].tile(
                            [P, 512], F32, tag=wo_tags[wi_ % 4],
                            name=f"wo_{tt0}_{oc}")
                        wi_ += 1
                        for h in range(HPC):
                            nc.tensor.matmul(ps[:],
                                             mm(outT[h][:, ttsl]),
                                             mm(wo_s[:, h, osl]),
                                             start=(h == 0),
                                             stop=(h == HPC - 1))
                        ev = ev_pool.tile([P, 512], FR, tag="ev",
                                          name=f"ev_{tt0}_{oc}")
                        # alternate the PSUM->SBUF eviction between DVE and
                        # ScalarE so neither engine eats the whole 88us;
                        # alternate the DRAM store across two DMA queues so
                        # the final chunk's 16 stores don't serialize on one
                        if wi_ % 2:
                            nc.scalar.copy(out=ev[:], in_=ps[:])
                        else:
                            nc.vector.tensor_copy(out=ev[:], in_=ps[:])
                        if wi_ % 2:
                            nc.gpsimd.dma_start(out=out[ttsl, osl], in_=ev[:])
                        else:
                            nc.sync.dma_start(out=out[ttsl, osl], in_=ev[:])

            pend = []
            for bb in range(b):
                for c in range(ich):
                    isl = slice(bb * s + c * tchunk, bb * s + (c + 1) * tchunk)
                    njt = r_mask * (c + 1)   # visible j-tiles
                    cs_ps = [p2cs.tile([P, tchunk], F32, tag=f"cs{h}",
                                       name=f"cs{h}_{bb}_{c}") for h in range(HPC)]
                    av_ps = [p2av.tile([P, tchunk], F32, tag=f"av{h}",
                                       name=f"av{h}_{bb}_{c}") for h in range(HPC)]
                    cctx = (bb, cs_ps, av_ps, isl, c)
                    for jt in range(njt):
                        jsl = slice(bb * s + jt * P, bb * s + (jt + 1) * P)
                        ri = jt - r_mask * c
                        lo = max(ri, 0) * P
                        w = tchunk - lo
                        csl = slice(isl.start + lo, isl.stop)
                        fl = dict(start=(jt == 0), stop=(jt == njt - 1))
                        sc = ab_pool.tile([P, HPC, 512], F32,
                                          tag=("A", "B")[jt % 2],
                                          name=f"sc_{bb}_{c}_{jt}")
                        for h in range(HPC):
                            nc.tensor.matmul(sc[:, h, :w], mm(kr[h][:, jsl]),
                                             mm(qr[h][:, csl]),
                                             start=True, stop=True)
                        es = exps_pool.tile([P, HPC, tchunk], FR, tag="es",
                                            name=f"es_{bb}_{c}_{jt}")
                        nc.scalar.activation(out=es[:, :, :w], in_=sc[:, :, :w],
                                             func=mybir.ActivationFunctionType.Exp,
                                             scale=float(SCALE))
                        if ri >= 0:  # diagonal-crossing tile
                            mb = mask_s[:, ri, lo:].unsqueeze(1).broadcast_to(
                                [P, HPC, w])
                            nc.vector.tensor_mul(out=es[:, :, :w],
                                                 in0=es[:, :, :w], in1=mb)
                        pend.append((es, lo, w, fl, jt, cctx))
                        if len(pend) > 2:
                            drain_one(pend)
            while pend:
                drain_one(pend)

    nc.compile()
    return nc


def make_in_maps(hidden_states, cos, sin, Wq, Wk, Wv, Wo, s=S, b=B, tchunk=512):
    t = b * s
    tchunk = min(tchunk, t)
    hs = np.asarray(hidden_states, np.float32).reshape(t, H)
    xt = np.ascontiguousarray(hs.T)
    cos2 = np.asarray(cos, np.float32).reshape(s, HD)
    sin2 = np.asarray(sin, np.float32).reshape(s, HD)
    cosP = np.ascontiguousarray(np.tile(cos2[:, PERM].T, (1, b))).astype(BF16)
    sign = np.where(PERM < 64, -1.0, 1.0).astype(np.float32)[:, None]
    sinP = np.ascontiguousarray(
        np.tile(sin2[:, PERM].T * sign, (1, b))).astype(BF16)
    masks_bf = build_masks(tchunk).astype(BF16)
    xt_bf = xt.astype(BF16)
    Wq, Wk, Wv, Wo = (np.asarray(w, np.float32) for w in (Wq, Wk, Wv, Wo))

    in_maps = []
    for c in range(NCORES):
        rows = np.concatenate([(HPC * c + hh) * HD + PERM for hh in range(HPC)])
        sl = slice(c * M, (c + 1) * M)
        in_maps.append({
            "xt": xt_bf,
            "wqt": np.ascontiguousarray(Wq[rows, :].T).astype(BF16),
            "wkt": np.ascontiguousarray(Wk[rows, :].T).astype(BF16),
            "wvt": np.ascontiguousarray(Wv[sl, :].T).astype(BF16),
            "wot": np.ascontiguousarray(Wo[:, sl].T).astype(BF16),
            "cost": cosP,
            "sint": sinP,
            "masks": masks_bf,
        })
    return in_maps


_CACHED_NC = None
_LAST_RESULTS = None


def kernel(hidden_states, cos, sin, Wq, Wk, Wv, Wo):
    global _CACHED_NC, _LAST_RESULTS
    in_maps = make_in_maps(hidden_states, cos, sin, Wq, Wk, Wv, Wo)
    if _CACHED_NC is None:
        _CACHED_NC = build_nc()
    res = run_bass_kernel_spmd(_CACHED_NC, in_maps, core_ids=list(range(NCORES)))
    _LAST_RESULTS = res
    acc = np.zeros((B * S, H), np.float32)
    for r in res.results:
        acc += r["out"].astype(np.float32)
    return acc.reshape(B, S, H)



# revision 13
# speedup vs baseline: 1.1296x; 1.0499x over previous
"""Trainium2 Bass kernel for nn_Attention_3126736192307 — merged pipeline.

Causal multi-head attention with RoPE: B=2, S=2048, H=2048, 16 heads x 128.

Sharding (tensor parallel over heads, 8 cores, 2 heads each):
  - Wq/Wk/Wv column-split (per-head), Wo row-split; each core computes a
    partial [B*S, H] output; the host sums the 8 partials.

v3: merged pipeline at tchunk=512. Chunk-step j emits
  proj(j) -> attn(j-1) -> rope(j)
so attention for chunk j-1 follows chunk j's projections in the PE FIFO
while j's RoPE (DVE) and PSUM evictions (ScalarE) run in their shadows.

PSUM discipline: a matmul accumulation group zeroes its whole 2KB bank at
start, so every concurrently-open group owns a bank. Projections run as
SEQUENTIAL groups (q_h0, q_h1, k_h0, k_h1, v0..v3) rotating 2 banks with
X.T chunk-resident in SBUF; attention processes one head at a time so a
single colsum + a single AV group are open at once (2+1 banks), and
score tiles rotate 3 banks shared with the Wo output tiles. 2+3+2+1 = 8.

Per-core dataflow (all matmuls transpose-free by construction):
  - Host pre-transposes: X.T [H, T], WqT/WkT [H, 256] (head-dim permuted so
    RoPE's rotate_half becomes an intra-quadrant stream_shuffle), WvT [H, 256],
    WoT [256, H], cos/sin [128, T] feature-major bf16 (sin sign-folded).
  - scores.T [j,i] = k.T (lhsT) @ q.T; exp on ScalarE (no max subtraction:
    scores ~N(0,1) after the 1/sqrt(hd) scale); causal block skipping + 0/1
    mask multiply on diagonal-crossing tiles; column sums via ones-matmul on
    TensorE; AV accumulation in PSUM; normalization folded into eviction.
  - out.T (lhsT) @ WoT -> partial [T, H]; PSUM->SBUF evictions alternate
    DVE/ScalarE and the DRAM stores alternate two DMA queues.
"""

import os
import sys

for _p in ("/opt/trn_rl_repo", "/root/.axon_site/_ro/trn_rl_repo"):
    if os.path.isdir(_p) and _p not in sys.path:
        sys.path.append(_p)

from contextlib import ExitStack

import ml_dtypes
import numpy as np

import concourse.bass as bass
import concourse.bacc as bacc
import concourse.tile as tile
from concourse import mybir
from concourse.bass_utils import run_bass_kernel_spmd

B, S, H, NH = 2, 2048, 2048, 16
HD = 128
NCORES = 8
HPC = NH // NCORES            # heads per core = 2
M = HPC * HD                  # 256 output channels per core
SCALE = HD ** -0.5
P = 128                       # partitions
NKT = H // P                  # 16 contraction tiles for projections

F32 = mybir.dt.float32

# head-dim permutation: interleave halves at 16 granularity so the RoPE
# partner (d <-> d+64) sits 16 partitions away inside one 32-part quadrant
PERM = np.concatenate([np.arange(16 * m, 16 * m + 16) + (64 if odd else 0)
                       for m in range(4) for odd in (0, 1)])
SWAP_MASK = [i ^ 16 for i in range(32)]

BF16 = ml_dtypes.bfloat16

TCHUNK = 512


def build_masks(tchunk):
    """0/1 keep-masks for the R diagonal-crossing j-tiles of each i-chunk."""
    r = tchunk // P
    m = np.zeros((r, P, tchunk), np.float32)
    il = np.arange(tchunk)
    for ri in range(r):
        for jl in range(P):
            m[ri, jl, :] = (P * ri + jl <= il).astype(np.float32)
    return m


def build_nc(s=S, b=B, tchunk=TCHUNK, mm_dtype=mybir.dt.bfloat16):
    t = b * s
    nch = t // tchunk             # 8 chunk-steps
    ich = s // tchunk             # 4 attention i-chunks per batch
    r_mask = tchunk // P          # 4 diagonal-crossing tiles per i-chunk
    ntt = t // P                  # 32 token tiles
    nvp = tchunk // P             # 4 v sub-tiles per chunk
    spt = s // P                  # 16 j-tiles per batch

    FR = mm_dtype
    EXP = mybir.ActivationFunctionType.Exp

    nc = bacc.Bacc("TRN2", target_bir_lowering=False, debug=False)

    xt = nc.declare_dram_parameter("xt", [H, t], FR, isOutput=False)
    wqt = nc.declare_dram_parameter("wqt", [H, M], FR, isOutput=False)
    wkt = nc.declare_dram_parameter("wkt", [H, M], FR, isOutput=False)
    wvt = nc.declare_dram_parameter("wvt", [H, M], FR, isOutput=False)
    wot = nc.declare_dram_parameter("wot", [M, H], FR, isOutput=False)
    cost = nc.declare_dram_parameter("cost", [HD, t], FR, isOutput=False)
    sint = nc.declare_dram_parameter("sint", [HD, t], FR, isOutput=False)
    masks = nc.declare_dram_parameter("masks", [r_mask, P, tchunk], FR,
                                      isOutput=False)
    out = nc.declare_dram_parameter("out", [t, H], FR, isOutput=True)

    with tile.TileContext(nc) as tc, ExitStack() as ctx:
        persist = ctx.enter_context(tc.tile_pool(name="persist", bufs=1))

        qr = [persist.tile([P, t], FR, tag=f"qr{h}", name=f"qr{h}")
              for h in range(HPC)]
        kr = [persist.tile([P, t], FR, tag=f"kr{h}", name=f"kr{h}")
              for h in range(HPC)]
        vv = persist.tile([P, ntt, M], FR, tag="vv")   # v[tt*128+p, d]
        ones_s = persist.tile([P, P], FR, tag="ones")
        nc.vector.memset(ones_s[:], 1.0)
        mask_s = persist.tile([P, r_mask, tchunk], FR, tag="masks")
        wo_s = persist.tile([P, HPC, H], FR, tag="wo")
        cos_sb = persist.tile([P, t], FR, tag="cosb")
        sin_sb = persist.tile([P, t], FR, tag="sinb")
        wq_s = persist.tile([P, NKT, M], FR, tag="wq")
        wk_s = persist.tile([P, NKT, M], FR, tag="wk")
        wv_s = persist.tile([P, NKT, M], FR, tag="wv")

        xt_pool = ctx.enter_context(tc.tile_pool(name="xtp", bufs=2))
        qkt_pool = ctx.enter_context(tc.tile_pool(name="qkt", bufs=2))
        rtmp_pool = ctx.enter_context(tc.tile_pool(name="rtmp", bufs=3))
        exps_pool = ctx.enter_context(tc.tile_pool(name="exps", bufs=6))
        rcp_pool = ctx.enter_context(tc.tile_pool(name="rcp", bufs=2))
        outT_pool = ctx.enter_context(tc.tile_pool(name="outT", bufs=2))
        ev_pool = ctx.enter_context(tc.tile_pool(name="evp", bufs=8))
        # PSUM: 8 banks: PR0 PR1 (sequential projection groups), S0 S1 S2
        # (score j-tiles + Wo outputs), C0 C1 (per-head colsums), AV
        pPR = ctx.enter_context(tc.tile_pool(name="pPR", bufs=1, space="PSUM"))
        pS = ctx.enter_context(tc.tile_pool(name="pS", bufs=1, space="PSUM"))
        pC = ctx.enter_context(tc.tile_pool(name="pC", bufs=1, space="PSUM"))
        pAV = ctx.enter_context(tc.tile_pool(name="pAV", bufs=1, space="PSUM"))

        # upfront loads on the gpsimd SWDGE queue in order of first use;
        # kt0 slivers first so the opening matmuls aren't gated on bulk DMAs
        for w_s, wsrc in ((wq_s, wqt), (wk_s, wkt), (wv_s, wvt)):
            nc.gpsimd.dma_start(
                out=w_s[:, 0:1, :],
                in_=wsrc[0:P, :].rearrange("(k p) m -> p k m", p=P))
        KWG = 5                     # weight k-tiles per bulk DMA (1+5+5+5)
        for lo_kt in range(1, NKT, KWG):
            hi_kt = min(lo_kt + KWG, NKT)
            gsl = slice(lo_kt * P, hi_kt * P)
            for w_s, wsrc in ((wq_s, wqt), (wk_s, wkt), (wv_s, wvt)):
                nc.gpsimd.dma_start(
                    out=w_s[:, lo_kt:hi_kt, :],
                    in_=wsrc[gsl, :].rearrange("(k p) m -> p k m", p=P))
        nc.gpsimd.dma_start(out=cos_sb[:], in_=cost[:, :])
        nc.gpsimd.dma_start(out=sin_sb[:], in_=sint[:, :])
        nc.gpsimd.dma_start(out=mask_s[:],
                            in_=masks.rearrange("r p n -> p r n"))
        nc.gpsimd.dma_start(out=wo_s[:],
                            in_=wot.rearrange("(mt p) o -> p mt o", p=P))

        pr_idx = [0]                  # rotating projection bank
        s_idx = [0]                   # rotating score/wo bank

        def emit_proj(j):
            tsl = slice(j * tchunk, (j + 1) * tchunk)
            # chunk-resident X.T: [P, NKT, tchunk] bf16 (16KB/partition)
            xt_s = xt_pool.tile([P, NKT, tchunk], FR, tag="xt",
                                name=f"xt{j}")
            if j == 0:
                # kt0 sliver first, then the bulk
                nc.sync.dma_start(
                    out=xt_s[:, 0:1, :],
                    in_=xt[0:P, tsl].rearrange("(k p) t -> p k t", p=P))
                for lo_kt in range(1, NKT, KWG):
                    hi_kt = min(lo_kt + KWG, NKT)
                    nc.sync.dma_start(
                        out=xt_s[:, lo_kt:hi_kt, :],
                        in_=xt[lo_kt * P:hi_kt * P, tsl].rearrange(
                            "(k p) t -> p k t", p=P))
            else:
                for lo_kt in range(0, NKT, 8):
                    nc.sync.dma_start(
                        out=xt_s[:, lo_kt:lo_kt + 8, :],
                        in_=xt[lo_kt * P:(lo_kt + 8) * P, tsl].rearrange(
                            "(k p) t -> p k t", p=P))
            qkt = qkt_pool.tile([P, 2, HPC, tchunk], FR, tag="qkt",
                                name=f"qkt{j}")
            # sequential accumulation groups, 2-bank rotation; evictions
            # (ScalarE q/k, DVE v) chase the groups
            for wi, w_s in ((0, wq_s), (1, wk_s)):
                for h in range(HPC):
                    ps = pPR.tile([P, tchunk], F32,
                                  tag=f"PR{pr_idx[0] % 2}",
                                  name=f"p{j}_{wi}_{h}")
                    pr_idx[0] += 1
                    msl = slice(h * P, (h + 1) * P)
                    for kt in range(NKT):
                        nc.tensor.matmul(ps[:], w_s[:, kt, msl],
                                         xt_s[:, kt, :],
                                         start=(kt == 0),
                                         stop=(kt == NKT - 1))
                    nc.scalar.copy(out=qkt[:, wi, h, :], in_=ps[:])
            for ts_ in range(nvp):
                ps = pPR.tile([P, tchunk], F32, tag=f"PR{pr_idx[0] % 2}",
                              name=f"pv{j}_{ts_}")
                pr_idx[0] += 1
                ssl = slice(ts_ * P, (ts_ + 1) * P)
                for kt in range(NKT):
                    nc.tensor.matmul(ps[:, :M], xt_s[:, kt, ssl],
                                     wv_s[:, kt, :],
                                     start=(kt == 0), stop=(kt == NKT - 1))
                nc.vector.tensor_copy(out=vv[:, j * nvp + ts_, :],
                                      in_=ps[:, :M])
            return qkt

        def emit_rope(j, qkt):
            tsl = slice(j * tchunk, (j + 1) * tchunk)
            for wi, dest in ((0, qr), (1, kr)):
                for h in range(HPC):
                    src = qkt[:, wi, h, :]
                    shuf = rtmp_pool.tile([P, tchunk], FR, tag="shuf")
                    dst = dest[h][:, tsl]
                    nc.vector.stream_shuffle(out=shuf[:], in_=src,
                                             mask=SWAP_MASK)
                    nc.vector.tensor_mul(out=dst, in0=src,
                                         in1=cos_sb[:, tsl])
                    nc.vector.tensor_mul(out=shuf[:], in0=shuf[:],
                                         in1=sin_sb[:, tsl])
                    nc.vector.tensor_add(out=dst, in0=dst, in1=shuf[:])

        def emit_attn(j):
            bb, c = divmod(j, ich)
            isl = slice(j * tchunk, (j + 1) * tchunk)
            njt = r_mask * (c + 1)
            outT = outT_pool.tile([P, HPC, tchunk], FR, tag="outT",
                                  name=f"oT{j}")
            for h in range(HPC):
                cs_ps = pC.tile([P, tchunk], F32, tag=f"C{h}",
                                name=f"cs{j}_{h}")
                av_ps = pAV.tile([P, tchunk], F32, tag="AV",
                                 name=f"av{j}_{h}")
                pend = []

                def drain_one():
                    es, plo, pw, pfl, pjt = pend.pop(0)
                    nc.tensor.matmul(cs_ps[:, plo:], ones_s[:],
                                     es[:, :pw], **pfl)
                    nc.tensor.matmul(av_ps[:, plo:],
                                     vv[:, bb * spt + pjt,
                                        h * P:(h + 1) * P],
                                     es[:, :pw], **pfl)

                for jt in range(njt):
                    jsl = slice(bb * s + jt * P, bb * s + (jt + 1) * P)
                    ri = jt - r_mask * c
                    lo = max(ri, 0) * P
                    w = tchunk - lo
                    csl = slice(isl.start + lo, isl.stop)
                    fl = dict(start=(jt == 0), stop=(jt == njt - 1))
                    sc = pS.tile([P, tchunk], F32, tag=f"S{s_idx[0] % 3}",
                                 name=f"sc{j}_{h}_{jt}")
                    s_idx[0] += 1
                    nc.tensor.matmul(sc[:, :w], kr[h][:, jsl],
                                     qr[h][:, csl], start=True, stop=True)
                    es = exps_pool.tile([P, tchunk], FR, tag="es",
                                        name=f"es{j}_{h}_{jt}")
                    nc.scalar.activation(out=es[:, :w], in_=sc[:, :w],
                                         func=EXP, scale=float(SCALE))
                    if ri >= 0:  # diagonal-crossing tile
                        nc.vector.tensor_mul(out=es[:, :w], in0=es[:, :w],
                                             in1=mask_s[:, ri, lo:])
                    pend.append((es, lo, w, fl, jt))
                    if len(pend) > 2:
                        drain_one()
                while pend:
                    drain_one()
                # normalize head h
                rcp = rcp_pool.tile([P, tchunk], F32, tag="rcp",
                                    name=f"rcp{j}_{h}")
                nc.vector.reciprocal_approx_fast(out=rcp[:], in_=cs_ps[:])
                nc.vector.tensor_mul(out=outT[:, h, :], in0=av_ps[:],
                                     in1=rcp[:])
            # output projection for the i-chunk
            for tt_ in range(tchunk // P):
                tt0 = isl.start + tt_ * P
                ttsl = slice(tt0, tt0 + P)
                for oc in range(H // 512):
                    osl = slice(oc * 512, (oc + 1) * 512)
                    ps = pS.tile([P, 512], F32, tag=f"S{s_idx[0] % 3}",
                                 name=f"wo{j}_{tt_}_{oc}")
                    s_idx[0] += 1
                    for h in range(HPC):
                        nc.tensor.matmul(
                            ps[:],
                            outT[:, h, tt_ * P:(tt_ + 1) * P],
                            wo_s[:, h, osl],
                            start=(h == 0), stop=(h == HPC - 1))
                    ev = ev_pool.tile([P, 512], FR, tag="ev",
                                      name=f"ev{j}_{tt_}_{oc}")
                    if s_idx[0] % 2:
                        nc.scalar.copy(out=ev[:], in_=ps[:])
                        nc.gpsimd.dma_start(out=out[ttsl, osl], in_=ev[:])
                    else:
                        nc.vector.tensor_copy(out=ev[:], in_=ps[:])
                        nc.sync.dma_start(out=out[ttsl, osl], in_=ev[:])

        for j in range(nch):
            qkt = emit_proj(j)
            if j > 0:
                emit_attn(j - 1)
            emit_rope(j, qkt)
        emit_attn(nch - 1)

    nc.compile()
    return nc


def make_in_maps(hidden_states, cos, sin, Wq, Wk, Wv, Wo, s=S, b=B,
                 tchunk=TCHUNK):
    t = b * s
    hs = np.asarray(hidden_states, np.float32).reshape(t, H)
    xt = np.ascontiguousarray(hs.T)
    cos2 = np.asarray(cos, np.float32).reshape(s, HD)
    sin2 = np.asarray(sin, np.float32).reshape(s, HD)
    cosP = np.ascontiguousarray(np.tile(cos2[:, PERM].T, (1, b))).astype(BF16)
    sign = np.where(PERM < 64, -1.0, 1.0).astype(np.float32)[:, None]
    sinP = np.ascontiguousarray(
        np.tile(sin2[:, PERM].T * sign, (1, b))).astype(BF16)
    masks_bf = build_masks(tchunk).astype(BF16)
    xt_bf = xt.astype(BF16)
    Wq, Wk, Wv, Wo = (np.asarray(w, np.float32) for w in (Wq, Wk, Wv, Wo))

    in_maps = []
    for c in range(NCORES):
        rows = np.concatenate([(HPC * c + hh) * HD + PERM
                               for hh in range(HPC)])
        sl = slice(c * M, (c + 1) * M)
        in_maps.append({
            "xt": xt_bf,
            "wqt": np.ascontiguousarray(Wq[rows, :].T).astype(BF16),
            "wkt": np.ascontiguousarray(Wk[rows, :].T).astype(BF16),
            "wvt": np.ascontiguousarray(Wv[sl, :].T).astype(BF16),
            "wot": np.ascontiguousarray(Wo[:, sl].T).astype(BF16),
            "cost": cosP,
            "sint": sinP,
            "masks": masks_bf,
        })
    return in_maps


_CACHED_NC = None
_LAST_RESULTS = None


def kernel(hidden_states, cos, sin, Wq, Wk, Wv, Wo):
    global _CACHED_NC, _LAST_RESULTS
    in_maps = make_in_maps(hidden_states, cos, sin, Wq, Wk, Wv, Wo)
    if _CACHED_NC is None:
        _CACHED_NC = build_nc()
    res = run_bass_kernel_spmd(_CACHED_NC, in_maps, core_ids=list(range(NCORES)))
    _LAST_RESULTS = res
    acc = np.zeros((B * S, H), np.float32)
    for r in res.results:
        acc += r["out"].astype(np.float32)
    return acc.reshape(B, S, H)
